# revision 27
# baseline (speedup 1.0000x reference)
"""Trainium2 Bass kernel for CachedMultiheadAttention (sliding-window + ALiBi).

Sharding: 8 cores = 2 batches x 4 head-quartets. Core c handles batch c//4 and
the 4 heads QUARTETS[c%4] (one steep + mid/shallow mix so the band-trimmed
attention work is balanced across cores while keeping a uniform SPMD program).
Each core computes the QKV projection for its heads, banded attention
(causal + 512 window + ALiBi), and a partial out-projection over its heads'
256 embedding columns. Host sums the 4 partials per batch.

On-chip pipeline (per core):
  x^T (f32r) --PE--> Q^T,K^T (bf16, [2heads*64, T] per pair) ; V is projected
  directly into natural layout [t,64] per 128-t block (xT-stationary matmuls),
  with a ones column at [...,64] (rowsum trick).
  S^T[j,q] = K^T.T @ Q^T per head slot, band width 128*NQ[slot] (trimmed for
  the steep slot: ALiBi decay makes far keys negligible), into a 2-bank PSUM
  tile so one exp covers up to 640 cols. P^T = exp(S^T) * bias tile (band
  mask + ALiBi, max-pre-shifted -> no online softmax) on ACT+DVE.
  AO^T[d,q] (+rowsum row) = V_aug.T @ P^T (M=65, accumulated over j blocks),
  normalized via reciprocal_approx_fast + gpsimd partition_broadcast; bf16
  out_proj per 512-q group, software-pipelined one group behind the AVs.
"""
import math

import numpy as np
import ml_dtypes

import concourse.bass as bass
import concourse.tile as tile
from concourse import bacc, mybir
from concourse.bass_utils import run_bass_kernel_spmd

F32 = mybir.dt.float32
F32R = mybir.dt.float32r
BF16 = mybir.dt.bfloat16

B, T, E, H, HD, W = 2, 2048, 1024, 16, 64, 512
NCORES = 8
NT = T // 128          # 16 key blocks of 128
NQ = [3, 5, 5, 5]      # score band width per head slot, in 128-col blocks
MB = [n - 1 for n in NQ]
BCOL = [n * 128 for n in NQ]
BOFF = [0, 384, 1024, 1664]
BTOT = 2304
# slot0: steep head (h0-h3), slot1: h4-h7, slots2-3: shallow h8-h15
QUARTETS = [[0, 4, 8, 9], [1, 5, 10, 11], [2, 6, 12, 13], [3, 7, 14, 15]]

_CACHE = {}


def _get_slopes(n):
    def p2(m):
        start = 2 ** (-(2 ** (-(math.log2(m) - 3))))
        return [start * start**i for i in range(m)]
    if math.log2(n) % 1 == 0:
        return p2(n)
    c = 2 ** math.floor(math.log2(n))
    return p2(c) + _get_slopes(2 * c)[0::2][: n - c]


def _build(dbg=False):
    nc = bacc.Bacc("TRN2", target_bir_lowering=False, debug=False, num_devices=NCORES)
    xT = nc.dram_tensor("xT", [8, 128, T], F32R, kind="ExternalInput").ap()
    wqkv = nc.dram_tensor("wqkv", [8, 128, 768], F32R, kind="ExternalInput").ap()
    wo = nc.dram_tensor("wo", [2, 128, E], BF16, kind="ExternalInput").ap()
    biasd = nc.dram_tensor("biasd", [128, BTOT], BF16, kind="ExternalInput").ap()
    outT = nc.dram_tensor("outT", [8, 128, T], BF16, kind="ExternalOutput").ap()
    if dbg:
        d_qkvT = nc.dram_tensor("d_qkvT", [128, 4, T], BF16, kind="ExternalOutput").ap()
        d_vnat = nc.dram_tensor("d_vnat", [128, NT, 4, HD], BF16, kind="ExternalOutput").ap()
        d_pth = nc.dram_tensor("d_pth", [128, NT, 640], BF16, kind="ExternalOutput").ap()
        d_ao2T = nc.dram_tensor("d_ao2T", [128, 2, T], BF16, kind="ExternalOutput").ap()
        d_rec4 = nc.dram_tensor("d_rec4", [128, 512], BF16, kind="ExternalOutput").ap()
        d_aop = nc.dram_tensor("d_aop", [128, 512], F32, kind="ExternalOutput").ap()

    with tile.TileContext(nc) as tc:
        with (
            tc.tile_pool(name="singles", bufs=1) as singles,
            tc.tile_pool(name="xp", bufs=2) as xp,
            tc.tile_pool(name="prp", bufs=3) as prp,
            tc.tile_pool(name="recp", bufs=2) as recp,
            tc.tile_pool(name="bcp", bufs=2) as bcp,
            tc.tile_pool(name="evp", bufs=3) as evp,
            tc.tile_pool(name="scp", bufs=2, space="PSUM") as scp,
            tc.tile_pool(name="auxp", bufs=4, space="PSUM") as auxp,
        ):
            # --- one-time loads (weights on gpsimd queue, x on SP) ---
            wqkv_sb = singles.tile([128, 8, 768], F32R)
            for ec in range(8):
                nc.gpsimd.dma_start(
                    wqkv_sb[:, ec, :], wqkv.rearrange("c p m -> p c m")[:, ec, :])
            wo_sb = singles.tile([128, 2, E], BF16)
            nc.gpsimd.dma_start(wo_sb[:], wo.rearrange("c p f -> p c f"))
            bias_sb = singles.tile([128, BTOT], BF16)
            nc.gpsimd.dma_start(bias_sb[:], biasd[:, :])

            qkvT = singles.tile([128, 4, T], BF16)   # slots: Qp0 Qp1 Kp0 Kp1
            vnat = singles.tile([128, NT, 4, HD], BF16)
            ones64 = singles.tile([128, HD], BF16)
            nc.gpsimd.memset(ones64[:], 1.0)
            zeros = singles.tile([128, 128], BF16)
            nc.gpsimd.memset(zeros[:], 0.0)
            ao2T = singles.tile([128, 2, T], BF16)   # normalized AO^T per pair
            pth = [singles.tile([128, NT, BCOL[s]], BF16, name=f"pth{s}")
                   for s in range(4)]

            # PE warm-up: dummy matmuls bridge the initial DMA window so HAM
            # un-throttles (1.2 -> 2.4 GHz) before the real work arrives.
            wup = scp.tile([128, 1024], F32, tag="sc")
            for i in range(40):
                nc.tensor.matmul(
                    wup[:, 0:128], lhsT=zeros[:, :], rhs=zeros[:, 0:128],
                    start=True, stop=True, skip_group_check=True,
                )

            def emit_scores_p(jb, p):
                """Scores + exp + bias-multiply for pair p of block jb."""
                for hh in range(2):
                    s = 2 * p + hh
                    w = min(NQ[s], NT - jb) * 128
                    w0 = min(w, 512)
                    r0 = hh * 64
                    sc = scp.tile([128, 1024], F32, tag="sc")
                    nc.tensor.matmul(
                        sc[:, 0:w0],
                        lhsT=qkvT[r0:r0 + 64, 2 + p, jb * 128:(jb + 1) * 128],
                        rhs=qkvT[r0:r0 + 64, p, jb * 128:jb * 128 + w0],
                        start=True, stop=True,
                    )
                    if w > 512:
                        nc.tensor.matmul(
                            sc[:, 512:w],
                            lhsT=qkvT[r0:r0 + 64, 2 + p, jb * 128:(jb + 1) * 128],
                            rhs=qkvT[r0:r0 + 64, p, jb * 128 + 512:jb * 128 + w],
                            start=True, stop=True,
                            skip_group_check=True,
                        )
                    praw = prp.tile([128, 640], BF16, tag="praw")
                    nc.scalar.activation(
                        out=praw[:, 0:w], in_=sc[:, 0:w],
                        func=mybir.ActivationFunctionType.Exp,
                    )
                    nc.vector.tensor_tensor(
                        out=pth[s][:, jb, 0:w], in0=praw[:, 0:w],
                        in1=bias_sb[:, BOFF[s]:BOFF[s] + w],
                        op=mybir.AluOpType.mult,
                    )

            # --- phase 1: QK^T projection (weight-stationary) + V natural
            # (xT-stationary). The prior chunk's score sets are woven between
            # matmul groups so each score's exp/mult drains while the PE runs
            # the next projection group (the 2-deep score-PSUM ring never
            # stalls the in-order PE queue). ---
            def emit_qk(tb, m, xc):
                pt = scp.tile([128, 1024], F32, tag="sc")
                for ec in range(8):
                    nc.tensor.matmul(
                        pt[:, 0:512],
                        lhsT=wqkv_sb[:, ec, m * 128:(m + 1) * 128],
                        rhs=xc[:, ec, :],
                        start=(ec == 0), stop=(ec == 7),
                    )
                dst = qkvT[:, m, tb * 512:(tb + 1) * 512]
                if m % 2 == 0:
                    nc.vector.tensor_copy(dst, pt[:, 0:512])
                else:
                    nc.scalar.copy(dst, pt[:, 0:512])

            def emit_vnat(tb, q, xc):
                jb = 4 * tb + q
                pv = auxp.tile([128, 4, HD], F32, tag="aux")
                for ec in range(8):
                    nc.tensor.matmul(
                        pv[:, :, :],
                        lhsT=xc[:, ec, q * 128:(q + 1) * 128],
                        rhs=wqkv_sb[:, ec, 512:768],
                        start=(ec == 0), stop=(ec == 7),
                    )
                nc.vector.tensor_copy(vnat[:, jb, :, :], pv[:, :, :])

            for tb in range(4):
                xc = xp.tile([128, 8, 512], F32R)
                for ec in range(8):
                    nc.sync.dma_start(
                        xc[:, ec, :], xT[ec, :, tb * 512:(tb + 1) * 512])
                emit_qk(tb, 0, xc)
                emit_qk(tb, 1, xc)
                groups = [lambda m=m: emit_qk(tb, m, xc) for m in (2, 3)]
                groups += [lambda q=q: emit_vnat(tb, q, xc) for q in range(4)]
                sets = ([(jb, p) for jb in range(4 * (tb - 1), 4 * tb)
                         for p in (0, 1)] if tb >= 1 else [])
                for i in range(max(len(groups), len(sets))):
                    if i < len(sets):
                        emit_scores_p(*sets[i])
                    if i < len(groups):
                        groups[i]()
            for jb in range(12, NT):
                emit_scores_p(jb, 0)
                emit_scores_p(jb, 1)

            if dbg:
                nc.sync.dma_start(d_pth[:], pth[1][:])

            # --- phase 2.5 + 3: AV + normalize per 512-q group, out_proj
            # software-pipelined one group behind ---
            def emit_outproj(tb):
                for fc in range(8):
                    po = scp.tile([128, 1024], F32, tag="sc")
                    for p in range(2):
                        nc.tensor.matmul(
                            po[:, 0:512],
                            lhsT=wo_sb[:, p, fc * 128:(fc + 1) * 128],
                            rhs=ao2T[:, p, tb * 512:(tb + 1) * 512],
                            start=(p == 0), stop=(p == 1),
                        )
                    ev = evp.tile([128, 512], BF16, tag="ev")
                    if fc % 2 == 0:
                        nc.scalar.copy(ev[:], po[:, 0:512])
                    else:
                        nc.vector.tensor_copy(ev[:], po[:, 0:512])
                    deng = nc.sync if fc % 2 == 0 else nc.gpsimd
                    deng.dma_start(outT[fc, :, tb * 512:(tb + 1) * 512], ev[:])

            def av_spans(g, s):
                """(jb, poff, ao_off, wdt) covering the band of query group g."""
                out = []
                for jb in range(max(0, 4 * g - MB[s]), 4 * g + 4):
                    qb_lo = max(4 * g, jb)
                    qb_hi = min(4 * g + 3, jb + MB[s])
                    if qb_hi < qb_lo:
                        continue
                    out.append((jb, (qb_lo - jb) * 128, (qb_lo - 4 * g) * 128,
                                (qb_hi - qb_lo + 1) * 128))
                return out

            for g in range(4):
                # rowsums first (independent of vnat): per pair, a zero-matmul
                # clears the bank's has_written, then each slot accumulates
                # 64-row REPLICATED rowsums (ones[128,64] lhsT) at col
                # positions {0,64} - 2-up concurrent, no awkward partitions.
                spans = [av_spans(g, s) for s in range(4)]
                nmax = max(len(sp) for sp in spans)
                rsps = []
                for p in range(2):
                    rsp = auxp.tile([128, 512], F32, tag="aux")
                    rsps.append(rsp)
                    nc.tensor.matmul(
                        rsp[:, :], lhsT=zeros[:, :], rhs=pth[1][:, 4 * g, 0:512],
                        start=True, stop=False, skip_group_check=True,
                    )
                for i in range(nmax):
                    for p in range(2):
                        for hh in range(2):
                            s = 2 * p + hh
                            if i >= len(spans[s]):
                                continue
                            jb, poff, ao_off, wdt = spans[s][i]
                            nc.tensor.matmul(
                                rsps[p][64 * hh:64 * hh + 64, ao_off:ao_off + wdt],
                                lhsT=ones64[:, :],
                                rhs=pth[s][:, jb, poff:poff + wdt],
                                start=False, stop=(i == len(spans[s]) - 1),
                                skip_group_check=True,
                            )
                # out_proj of the previous group slots in here: its matmuls
                # only need g-1's normalized ao2T, keeping the PE busy while
                # this group's normalization chain drains on ACT/DVE
                if g >= 1:
                    emit_outproj(g - 1)
                # AV pair-packed: slot 2p -> rows 0:64, slot 2p+1 -> rows 64:128
                aops = []
                for p in range(2):
                    aot = auxp.tile([128, 512], F32, tag="aux")
                    aops.append(aot)
                    for hh in range(2):
                        s = 2 * p + hh
                        for i, (jb, poff, ao_off, wdt) in enumerate(spans[s]):
                            nc.tensor.matmul(
                                aot[64 * hh:64 * hh + 64, ao_off:ao_off + wdt],
                                lhsT=vnat[:, jb, s, 0:HD],
                                rhs=pth[s][:, jb, poff:poff + wdt],
                                start=(i == 0), stop=(i == len(spans[s]) - 1),
                                skip_group_check=True,
                            )
                # reciprocal over the whole tile: only rows 0/32/64/96 are
                # meaningful (DVE cost is free-dim only, idle lanes are free)
                if dbg and g == 0:
                    dcr = evp.tile([128, 512], BF16, tag="dcr")
                    nc.vector.tensor_copy(dcr[:], rsps[0][:, :])
                    nc.sync.dma_start(d_rec4[:], dcr[:])
                    dcp = evp.tile([128, 512], F32, tag="dcp")
                    nc.vector.tensor_copy(dcp[:], aops[0][:, :])
                    nc.sync.dma_start(d_aop[:], dcp[:])
                # normalize: reciprocal of the replicated-rowsum tile.
                # p=0 via ACT exp(-ln) (same activation table set as Exp),
                # p=1 via DVE reciprocal - balances the two engines.
                for p in range(2):
                    bc = bcp.tile([128, 512], F32, tag="bc")
                    if p == 0:
                        lntmp = recp.tile([128, 512], F32, tag="rec")
                        nc.scalar.activation(
                            out=lntmp[:], in_=rsps[p][:, :],
                            func=mybir.ActivationFunctionType.Ln)
                        nc.scalar.activation(
                            out=bc[:], in_=lntmp[:],
                            func=mybir.ActivationFunctionType.Exp, scale=-1.0)
                    else:
                        nc.vector.reciprocal(bc[:], rsps[p][:, :])
                    nc.vector.tensor_tensor(
                        out=ao2T[:, p, g * 512:(g + 1) * 512],
                        in0=aops[p][:, :], in1=bc[:], op=mybir.AluOpType.mult,
                    )
            emit_outproj(3)

            if dbg:
                nc.sync.dma_start(d_qkvT[:], qkvT[:])
                nc.sync.dma_start(d_vnat[:], vnat[:])
                nc.sync.dma_start(d_ao2T[:], ao2T[:])

    nc.compile()
    return nc


def _host_inputs(query, in_proj_weight, out_proj_weight):
    """Per-core input maps (numpy only)."""
    slopes = np.asarray(_get_slopes(H), np.float32)
    q32 = np.asarray(query, np.float32)
    w_in = np.asarray(in_proj_weight, np.float32)
    w_out = np.asarray(out_proj_weight, np.float32)

    jj = np.arange(128)[:, None]
    in_maps = []
    for c in range(NCORES):
        b, qr = divmod(c, 4)
        heads = QUARTETS[qr]
        rows = np.concatenate([h * HD + np.arange(HD) for h in heads])  # 256
        wq = w_in[rows, :] * (1.0 / math.sqrt(HD))
        wk = w_in[E + rows, :]
        wv = w_in[2 * E + rows, :]
        w_loc = np.concatenate([wq, wk, wv], axis=0)          # [768, E]
        wqkv_a = np.ascontiguousarray(w_loc.T.reshape(8, 128, 768), np.float32)

        xTa = np.ascontiguousarray(q32[b].T.reshape(8, 128, T), np.float32)

        wo_loc = np.ascontiguousarray(
            w_out[:, rows].T.reshape(2, 128, E)).astype(ml_dtypes.bfloat16)

        biasd = np.zeros((128, BTOT), ml_dtypes.bfloat16)
        for s in range(4):
            sl = slopes[heads[s]]
            cc = np.arange(BCOL[s])[None, :]
            allowed = (cc >= jj) & (cc - jj <= W)
            eb = np.where(allowed, np.exp(-sl * (cc - jj).astype(np.float64)), 0.0)
            biasd[:, BOFF[s]:BOFF[s] + BCOL[s]] = eb.astype(ml_dtypes.bfloat16)

        in_maps.append({"xT": xTa, "wqkv": wqkv_a, "wo": wo_loc, "biasd": biasd})
    return in_maps


def _assemble(results):
    out = np.zeros((B, T, E), np.float32)
    for c in range(NCORES):
        b = c // 4
        part = np.asarray(results[c]["outT"]).astype(np.float32)  # [8,128,T]
        out[b] += part.reshape(E, T).T
    return out


def kernel(query, in_proj_weight, out_proj_weight, num_heads, sliding_window_size):
    assert int(num_heads) == H and int(sliding_window_size) == W
    assert query.shape == (B, T, E)
    if "nc" not in _CACHE:
        _CACHE["nc"] = _build()
    in_maps = _host_inputs(query, in_proj_weight, out_proj_weight)
    res = run_bass_kernel_spmd(_CACHE["nc"], in_maps, list(range(NCORES))).results
    return _assemble(res)


# revision 28
# speedup vs baseline: 1.0815x; 1.0815x over previous
"""Trainium2 Bass kernel for CachedMultiheadAttention (sliding-window + ALiBi).

Sharding: 8 cores = 2 batches x 4 head-quartets. Core c handles batch c//4 and
the 4 heads QUARTETS[c%4] (one steep + mid/shallow mix so the band-trimmed
attention work is balanced across cores while keeping a uniform SPMD program).
Each core computes the QKV projection for its heads, banded attention
(causal + 512 window + ALiBi), and a partial out-projection over its heads'
256 embedding columns. Host sums the 4 partials per batch.

On-chip pipeline (per core):
  x^T (f32r) --PE--> Q^T,K^T (bf16, [2heads*64, T] per pair) ; V is projected
  directly into natural layout [t,64] per 128-t block (xT-stationary matmuls),
  with a ones column at [...,64] (rowsum trick).
  S^T[j,q] = K^T.T @ Q^T per head slot, band width 128*NQ[slot] (trimmed for
  the steep slot: ALiBi decay makes far keys negligible), into a 2-bank PSUM
  tile so one exp covers up to 640 cols. P^T = exp(S^T) * bias tile (band
  mask + ALiBi, max-pre-shifted -> no online softmax) on ACT+DVE.
  AO^T[d,q] (+rowsum row) = V_aug.T @ P^T (M=65, accumulated over j blocks),
  normalized via reciprocal_approx_fast + gpsimd partition_broadcast; bf16
  out_proj per 512-q group, software-pipelined one group behind the AVs.
"""
import math

import numpy as np
import ml_dtypes

import concourse.bass as bass
import concourse.tile as tile
from concourse import bacc, mybir
from concourse.bass_utils import run_bass_kernel_spmd

F32 = mybir.dt.float32
F32R = mybir.dt.float32r
BF16 = mybir.dt.bfloat16

B, T, E, H, HD, W = 2, 2048, 1024, 16, 64, 512
NCORES = 8
NT = T // 128          # 16 key blocks of 128
NQ = [3, 5, 5, 5]      # score band width per head slot, in 128-col blocks
MB = [n - 1 for n in NQ]
BCOL = [n * 128 for n in NQ]
BOFF = [0, 384, 1024, 1664]
BTOT = 2304
# slot0: steep head (h0-h3), slot1: h4-h7, slots2-3: shallow h8-h15
QUARTETS = [[0, 4, 8, 9], [1, 5, 10, 11], [2, 6, 12, 13], [3, 7, 14, 15]]

_CACHE = {}


def _get_slopes(n):
    def p2(m):
        start = 2 ** (-(2 ** (-(math.log2(m) - 3))))
        return [start * start**i for i in range(m)]
    if math.log2(n) % 1 == 0:
        return p2(n)
    c = 2 ** math.floor(math.log2(n))
    return p2(c) + _get_slopes(2 * c)[0::2][: n - c]


def _build(dbg=False):
    nc = bacc.Bacc("TRN2", target_bir_lowering=False, debug=False, num_devices=NCORES)
    xT = nc.dram_tensor("xT", [8, 128, T], F32R, kind="ExternalInput").ap()
    wqkv = nc.dram_tensor("wqkv", [8, 128, 768], F32R, kind="ExternalInput").ap()
    wo = nc.dram_tensor("wo", [2, 128, E], BF16, kind="ExternalInput").ap()
    biasd = nc.dram_tensor("biasd", [128, BTOT], BF16, kind="ExternalInput").ap()
    outT = nc.dram_tensor("outT", [8, 128, T], BF16, kind="ExternalOutput").ap()
    if dbg:
        d_qkvT = nc.dram_tensor("d_qkvT", [128, 4, T], BF16, kind="ExternalOutput").ap()
        d_vnat = nc.dram_tensor("d_vnat", [128, NT, 4, HD], BF16, kind="ExternalOutput").ap()
        d_pth = nc.dram_tensor("d_pth", [128, NT, 640], BF16, kind="ExternalOutput").ap()
        d_ao2T = nc.dram_tensor("d_ao2T", [128, 2, T], BF16, kind="ExternalOutput").ap()
        d_rec4 = nc.dram_tensor("d_rec4", [128, 512], BF16, kind="ExternalOutput").ap()
        d_aop = nc.dram_tensor("d_aop", [128, 512], F32, kind="ExternalOutput").ap()

    with tile.TileContext(nc) as tc:
        with (
            tc.tile_pool(name="singles", bufs=1) as singles,
            tc.tile_pool(name="xp", bufs=2) as xp,
            tc.tile_pool(name="prp", bufs=3) as prp,
            tc.tile_pool(name="recp", bufs=2) as recp,
            tc.tile_pool(name="bcp", bufs=2) as bcp,
            tc.tile_pool(name="evp", bufs=3) as evp,
            tc.tile_pool(name="scp", bufs=2, space="PSUM") as scp,
            tc.tile_pool(name="auxp", bufs=4, space="PSUM") as auxp,
        ):
            # constants first: their memsets must not queue behind the
            # weight-DMA issues on gpsimd (the warm-up matmuls need them)
            ones64 = singles.tile([128, HD], BF16)
            nc.gpsimd.memset(ones64[:], 1.0)
            zeros = singles.tile([128, 128], BF16)
            nc.gpsimd.memset(zeros[:], 0.0)

            # --- one-time loads (weights on gpsimd queue, x on SP) ---
            wqkv_sb = singles.tile([128, 8, 768], F32R)
            for ec in range(8):
                nc.gpsimd.dma_start(
                    wqkv_sb[:, ec, :], wqkv.rearrange("c p m -> p c m")[:, ec, :])
            wo_sb = singles.tile([128, 2, E], BF16)
            nc.gpsimd.dma_start(wo_sb[:], wo.rearrange("c p f -> p c f"))
            bias_sb = singles.tile([128, BTOT], BF16)
            nc.gpsimd.dma_start(bias_sb[:], biasd[:, :])

            qkvT = singles.tile([128, 4, T], BF16)   # slots: Qp0 Qp1 Kp0 Kp1
            vnat = singles.tile([128, NT, 4, HD], BF16)
            ao2T = singles.tile([128, 2, T], BF16)   # normalized AO^T per pair
            pth = [singles.tile([128, NT, BCOL[s]], BF16, name=f"pth{s}")
                   for s in range(4)]

            # PE warm-up: dummy matmuls bridge the initial DMA window so HAM
            # un-throttles (1.2 -> 2.4 GHz) before the real work arrives.
            wup = scp.tile([128, 1024], F32, tag="sc")
            for i in range(40):
                nc.tensor.matmul(
                    wup[:, 0:128], lhsT=zeros[:, :], rhs=zeros[:, 0:128],
                    start=True, stop=True, skip_group_check=True,
                )

            def emit_scores_p(jb, p):
                """Scores + exp + bias-multiply for pair p of block jb."""
                for hh in range(2):
                    s = 2 * p + hh
                    w = min(NQ[s], NT - jb) * 128
                    w0 = min(w, 512)
                    r0 = hh * 64
                    sc = scp.tile([128, 1024], F32, tag="sc")
                    nc.tensor.matmul(
                        sc[:, 0:w0],
                        lhsT=qkvT[r0:r0 + 64, 2 + p, jb * 128:(jb + 1) * 128],
                        rhs=qkvT[r0:r0 + 64, p, jb * 128:jb * 128 + w0],
                        start=True, stop=True,
                    )
                    if w > 512:
                        nc.tensor.matmul(
                            sc[:, 512:w],
                            lhsT=qkvT[r0:r0 + 64, 2 + p, jb * 128:(jb + 1) * 128],
                            rhs=qkvT[r0:r0 + 64, p, jb * 128 + 512:jb * 128 + w],
                            start=True, stop=True,
                            skip_group_check=True,
                        )
                    praw = prp.tile([128, 640], BF16, tag="praw")
                    nc.scalar.activation(
                        out=praw[:, 0:w], in_=sc[:, 0:w],
                        func=mybir.ActivationFunctionType.Exp,
                    )
                    nc.vector.tensor_tensor(
                        out=pth[s][:, jb, 0:w], in0=praw[:, 0:w],
                        in1=bias_sb[:, BOFF[s]:BOFF[s] + w],
                        op=mybir.AluOpType.mult,
                    )

            # --- phase 1: QK^T projection (weight-stationary) + V natural
            # (xT-stationary). The prior chunk's score sets are woven between
            # matmul groups so each score's exp/mult drains while the PE runs
            # the next projection group (the 2-deep score-PSUM ring never
            # stalls the in-order PE queue). ---
            def emit_qk(tb, m, xc):
                pt = scp.tile([128, 1024], F32, tag="sc")
                for ec in range(8):
                    nc.tensor.matmul(
                        pt[:, 0:512],
                        lhsT=wqkv_sb[:, ec, m * 128:(m + 1) * 128],
                        rhs=xc[:, ec, :],
                        start=(ec == 0), stop=(ec == 7),
                    )
                dst = qkvT[:, m, tb * 512:(tb + 1) * 512]
                nc.vector.tensor_copy(dst, pt[:, 0:512])

            def emit_vnat(tb, q, xc):
                jb = 4 * tb + q
                pv = auxp.tile([128, 4, HD], F32, tag="aux")
                for ec in range(8):
                    nc.tensor.matmul(
                        pv[:, :, :],
                        lhsT=xc[:, ec, q * 128:(q + 1) * 128],
                        rhs=wqkv_sb[:, ec, 512:768],
                        start=(ec == 0), stop=(ec == 7),
                    )
                nc.vector.tensor_copy(vnat[:, jb, :, :], pv[:, :, :])

            for tb in range(4):
                xc = xp.tile([128, 8, 512], F32R)
                for ec in range(8):
                    nc.sync.dma_start(
                        xc[:, ec, :], xT[ec, :, tb * 512:(tb + 1) * 512])
                emit_qk(tb, 0, xc)
                emit_qk(tb, 1, xc)
                groups = [lambda m=m: emit_qk(tb, m, xc) for m in (2, 3)]
                groups += [lambda q=q: emit_vnat(tb, q, xc) for q in range(4)]
                sets = ([(jb, p) for jb in range(4 * (tb - 1), 4 * tb)
                         for p in (0, 1)] if tb >= 1 else [])
                for i in range(max(len(groups), len(sets))):
                    if i < len(sets):
                        emit_scores_p(*sets[i])
                    if i < len(groups):
                        groups[i]()
            for jb in range(12, NT):
                emit_scores_p(jb, 0)
                emit_scores_p(jb, 1)

            if dbg:
                nc.sync.dma_start(d_pth[:], pth[1][:])

            # --- phase 2.5 + 3: AV + normalize per 512-q group, out_proj
            # software-pipelined one group behind ---
            def emit_outproj(tb):
                for fc in range(8):
                    po = scp.tile([128, 1024], F32, tag="sc")
                    for p in range(2):
                        nc.tensor.matmul(
                            po[:, 0:512],
                            lhsT=wo_sb[:, p, fc * 128:(fc + 1) * 128],
                            rhs=ao2T[:, p, tb * 512:(tb + 1) * 512],
                            start=(p == 0), stop=(p == 1),
                        )
                    ev = evp.tile([128, 512], BF16, tag="ev")
                    nc.scalar.copy(ev[:], po[:, 0:512])
                    deng = nc.sync if fc % 2 == 0 else nc.gpsimd
                    deng.dma_start(outT[fc, :, tb * 512:(tb + 1) * 512], ev[:])

            def av_spans(g, s):
                """(jb, poff, ao_off, wdt) covering the band of query group g."""
                out = []
                for jb in range(max(0, 4 * g - MB[s]), 4 * g + 4):
                    qb_lo = max(4 * g, jb)
                    qb_hi = min(4 * g + 3, jb + MB[s])
                    if qb_hi < qb_lo:
                        continue
                    out.append((jb, (qb_lo - jb) * 128, (qb_lo - 4 * g) * 128,
                                (qb_hi - qb_lo + 1) * 128))
                return out

            for g in range(4):
                # rowsums first (independent of vnat): per pair, a zero-matmul
                # clears the bank's has_written, then each slot accumulates
                # 64-row REPLICATED rowsums (ones[128,64] lhsT) at col
                # positions {0,64} - 2-up concurrent, no awkward partitions.
                spans = [av_spans(g, s) for s in range(4)]
                nmax = max(len(sp) for sp in spans)
                rsps = []
                for p in range(2):
                    rsp = auxp.tile([128, 512], F32, tag="aux")
                    rsps.append(rsp)
                    nc.tensor.matmul(
                        rsp[:, :], lhsT=zeros[:, :], rhs=pth[1][:, 4 * g, 0:512],
                        start=True, stop=False, skip_group_check=True,
                    )
                for i in range(nmax):
                    for p in range(2):
                        for hh in range(2):
                            s = 2 * p + hh
                            if i >= len(spans[s]):
                                continue
                            jb, poff, ao_off, wdt = spans[s][i]
                            nc.tensor.matmul(
                                rsps[p][64 * hh:64 * hh + 64, ao_off:ao_off + wdt],
                                lhsT=ones64[:, :],
                                rhs=pth[s][:, jb, poff:poff + wdt],
                                start=False, stop=(i == len(spans[s]) - 1),
                                skip_group_check=True,
                            )
                # out_proj of the previous group slots in here: its matmuls
                # only need g-1's normalized ao2T, keeping the PE busy while
                # this group's normalization chain drains on ACT/DVE
                if g >= 1:
                    emit_outproj(g - 1)
                # AV pair-packed: slot 2p -> rows 0:64, slot 2p+1 -> rows 64:128
                aops = []
                for p in range(2):
                    aot = auxp.tile([128, 512], F32, tag="aux")
                    aops.append(aot)
                    for hh in range(2):
                        s = 2 * p + hh
                        for i, (jb, poff, ao_off, wdt) in enumerate(spans[s]):
                            nc.tensor.matmul(
                                aot[64 * hh:64 * hh + 64, ao_off:ao_off + wdt],
                                lhsT=vnat[:, jb, s, 0:HD],
                                rhs=pth[s][:, jb, poff:poff + wdt],
                                start=(i == 0), stop=(i == len(spans[s]) - 1),
                                skip_group_check=True,
                            )
                # reciprocal over the whole tile: only rows 0/32/64/96 are
                # meaningful (DVE cost is free-dim only, idle lanes are free)
                if dbg and g == 0:
                    dcr = evp.tile([128, 512], BF16, tag="dcr")
                    nc.vector.tensor_copy(dcr[:], rsps[0][:, :])
                    nc.sync.dma_start(d_rec4[:], dcr[:])
                    dcp = evp.tile([128, 512], F32, tag="dcp")
                    nc.vector.tensor_copy(dcp[:], aops[0][:, :])
                    nc.sync.dma_start(d_aop[:], dcp[:])
                # normalize: reciprocal of the replicated-rowsum tile.
                # p=0 via ACT exp(-ln) (same activation table set as Exp),
                # p=1 via DVE reciprocal - balances the two engines.
                for p in range(2):
                    bc = bcp.tile([128, 512], F32, tag="bc")
                    if p == 0:
                        lntmp = recp.tile([128, 512], F32, tag="rec")
                        nc.scalar.activation(
                            out=lntmp[:], in_=rsps[p][:, :],
                            func=mybir.ActivationFunctionType.Ln)
                        nc.scalar.activation(
                            out=bc[:], in_=lntmp[:],
                            func=mybir.ActivationFunctionType.Exp, scale=-1.0)
                    else:
                        nc.vector.reciprocal(bc[:], rsps[p][:, :])
                    nc.vector.tensor_tensor(
                        out=ao2T[:, p, g * 512:(g + 1) * 512],
                        in0=aops[p][:, :], in1=bc[:], op=mybir.AluOpType.mult,
                    )
            emit_outproj(3)

            if dbg:
                nc.sync.dma_start(d_qkvT[:], qkvT[:])
                nc.sync.dma_start(d_vnat[:], vnat[:])
                nc.sync.dma_start(d_ao2T[:], ao2T[:])

    nc.compile()
    return nc


def _host_inputs(query, in_proj_weight, out_proj_weight):
    """Per-core input maps (numpy only)."""
    slopes = np.asarray(_get_slopes(H), np.float32)
    q32 = np.asarray(query, np.float32)
    w_in = np.asarray(in_proj_weight, np.float32)
    w_out = np.asarray(out_proj_weight, np.float32)

    jj = np.arange(128)[:, None]
    in_maps = []
    for c in range(NCORES):
        b, qr = divmod(c, 4)
        heads = QUARTETS[qr]
        rows = np.concatenate([h * HD + np.arange(HD) for h in heads])  # 256
        wq = w_in[rows, :] * (1.0 / math.sqrt(HD))
        wk = w_in[E + rows, :]
        wv = w_in[2 * E + rows, :]
        w_loc = np.concatenate([wq, wk, wv], axis=0)          # [768, E]
        wqkv_a = np.ascontiguousarray(w_loc.T.reshape(8, 128, 768), np.float32)

        xTa = np.ascontiguousarray(q32[b].T.reshape(8, 128, T), np.float32)

        wo_loc = np.ascontiguousarray(
            w_out[:, rows].T.reshape(2, 128, E)).astype(ml_dtypes.bfloat16)

        biasd = np.zeros((128, BTOT), ml_dtypes.bfloat16)
        for s in range(4):
            sl = slopes[heads[s]]
            cc = np.arange(BCOL[s])[None, :]
            allowed = (cc >= jj) & (cc - jj <= W)
            eb = np.where(allowed, np.exp(-sl * (cc - jj).astype(np.float64)), 0.0)
            biasd[:, BOFF[s]:BOFF[s] + BCOL[s]] = eb.astype(ml_dtypes.bfloat16)

        in_maps.append({"xT": xTa, "wqkv": wqkv_a, "wo": wo_loc, "biasd": biasd})
    return in_maps


def _assemble(results):
    out = np.zeros((B, T, E), np.float32)
    for c in range(NCORES):
        b = c // 4
        part = np.asarray(results[c]["outT"]).astype(np.float32)  # [8,128,T]
        out[b] += part.reshape(E, T).T
    return out


def kernel(query, in_proj_weight, out_proj_weight, num_heads, sliding_window_size):
    assert int(num_heads) == H and int(sliding_window_size) == W
    assert query.shape == (B, T, E)
    if "nc" not in _CACHE:
        _CACHE["nc"] = _build()
    in_maps = _host_inputs(query, in_proj_weight, out_proj_weight)
    res = run_bass_kernel_spmd(_CACHE["nc"], in_maps, list(range(NCORES))).results
    return _assemble(res)
